# revision 20
# baseline (speedup 1.0000x reference)
"""DeepSeek-style GQA attention block (B=2, S=2048, H=1536, 12 q-heads /
2 kv-heads, d=128) sharded over 8 TRN2 NeuronCores.

Sharding: core = (batch b, kv-group hh, query-half th).
  - tensor parallel over the 2 kv groups (6 q-heads + 1 kv head each)
  - data parallel over batch (2)
  - query-token parallel (2 halves of 1024)
Each core computes its 6 heads' attention for its 1024 query tokens against
the full 2048-token K/V of its kv head, then a partial O-projection; the two
kv-group partials per (b, th) are summed on the host.

All matmuls run in bf16 with fp32 PSUM accumulation. Softmax runs without
max-subtraction (scores are O(1) here), with the 1/sqrt(d) scale and the
additive attention-mask bias fused into the ACT exp instruction.

Layout trick: scores are computed TRANSPOSED (scores^T[Sk, Sq] = K Q^T) so
that the attention probabilities come out with Sk on partitions, which is
exactly the layout the AV matmul needs as its moving operand — no on-chip
transpose of the [Sk, Sq] probability matrix is ever needed. The softmax
denominator (a partition-dim reduction) is computed with ones-vector matmuls
on the tensor engine, and normalization is applied to the tiny AV output
(out^T[d, Sq]) via a PE rank-1 broadcast + DVE reciprocal/multiply.
"""

import numpy as np
import ml_dtypes

HIDDEN = 1536
D = 128          # head dim
NH = 6           # q-heads per core
B, S = 2, 2048
SQ = 1024        # query tokens per core
HC = HIDDEN // 128   # 12 hidden chunks
SKC = S // 128       # 16 key chunks
SCALE = float(1.0 / np.sqrt(np.float32(D)))

_NC_CACHE = {}
last_results = None  # BassKernelResults of the most recent run (for test.py)


def _build_nc():
    import concourse.bacc as bacc
    import concourse.mybir as mybir
    import concourse.tile as tile
    from concourse.masks import make_identity

    bf16 = mybir.dt.bfloat16
    f32 = mybir.dt.float32
    f32r = mybir.dt.float32r
    Exp = mybir.ActivationFunctionType.Exp

    nc = bacc.Bacc("TRN2", target_bir_lowering=False, debug=False, num_devices=8)

    xt = nc.dram_tensor("xt", [HIDDEN, S], bf16, kind="ExternalInput")
    wqt = nc.dram_tensor("wqt", [HIDDEN, NH * D], bf16, kind="ExternalInput")
    wkt = nc.dram_tensor("wkt", [HIDDEN, D], bf16, kind="ExternalInput")
    wvt = nc.dram_tensor("wvt", [HIDDEN, D], bf16, kind="ExternalInput")
    wot = nc.dram_tensor("wot", [NH * D, HIDDEN], bf16, kind="ExternalInput")
    biasd = nc.dram_tensor("biasd", [128, SKC], f32, kind="ExternalInput")
    y = nc.dram_tensor("y", [SQ, HIDDEN], f32, kind="ExternalOutput")

    with tile.TileContext(nc) as tc:
        with (
            tc.tile_pool(name="const", bufs=1) as constp,
            tc.tile_pool(name="weights", bufs=1) as wp,
            tc.tile_pool(name="persist", bufs=1) as pers,
        ):
            ident = constp.tile([128, 128], bf16)
            make_identity(nc, ident[:])
            ones_col = constp.tile([128, 1], bf16)
            nc.vector.memset(ones_col[:], 1.0)
            ones_row = constp.tile([1, 128], bf16)
            nc.vector.memset(ones_row[:], 1.0)
            bias_sb = constp.tile([128, SKC], f32)
            nc.sync.dma_start(bias_sb[:], biasd.ap())

            # wk/wv/xt DMAs are emitted first; the big wq/wo loads are
            # deferred until after the K/V matmul stream is emitted so they
            # don't steal DMA bandwidth from the xt chunks that gate the
            # first matmuls.
            wq_sb = wp.tile([128, HC, NH * D], bf16)
            wk_sb = wp.tile([128, HC, D], bf16)
            nc.sync.dma_start(wk_sb[:], wkt.ap().rearrange("(c p) m -> p c m", p=128))
            wv_sb = wp.tile([128, HC, D], bf16)
            nc.sync.dma_start(wv_sb[:], wvt.ap().rearrange("(c p) m -> p c m", p=128))
            wo_sb = wp.tile([128, NH, HIDDEN], bf16)

            kT_sb = pers.tile([128, S], bf16)         # K^T [d, Sk]
            v_sb = pers.tile([128, SKC, D], bf16)     # V [Sk, d], chunked
            qT_sb = pers.tile([128, NH, SQ], bf16)    # Q^T [d, Sq] per head
            outT_sb = pers.tile([128, NH, SQ], bf16)  # AV out^T [d, Sq] per head

            # ---------- phase 1-3: projections ----------
            with (
                tc.tile_pool(name="xtp", bufs=1) as xtp,
                tc.tile_pool(name="vtp", bufs=1) as vtp,
            ):
                # xt and wq stream chunk-by-chunk, interleaved, so projection
                # matmuls pipeline against DMA arrival instead of waiting for
                # the full tensors.
                xt_sb = xtp.tile([128, HC, S], bf16)
                wqt_r = wqt.ap().rearrange("(c p) m -> p c m", p=128)
                for c in range(HC):
                    nc.sync.dma_start(xt_sb[:, c, :], xt[128 * c : 128 * (c + 1), :])
                    nc.sync.dma_start(wq_sb[:, c, :], wqt_r[:, c, :])

                # K^T and V^T = W X^T: all four 512-wide Sk blocks of each
                # accumulate chunk-major, so every xt chunk arrival feeds 8
                # matmuls immediately (needs all 8 PSUM banks; the pool is
                # closed before the transpose/Q pool opens).
                vT_sb = vtp.tile([128, S], bf16)
                with tc.tile_pool(name="kv_ps", bufs=1, space="PSUM") as kvps:
                    kps = kvps.tile([128, 4, 512], f32, tag="kps")
                    vps = kvps.tile([128, 4, 512], f32, tag="vps")
                    for c in range(HC):
                        for w_sb, ps in ((wk_sb, kps), (wv_sb, vps)):
                            for sb in range(S // 512):
                                nc.tensor.matmul(
                                    ps[:, sb, :],
                                    w_sb[:, c, :],
                                    xt_sb[:, c, 512 * sb : 512 * (sb + 1)],
                                    start=(c == 0),
                                    stop=(c == HC - 1),
                                )
                    for ps, dst in ((kps, kT_sb), (vps, vT_sb)):
                        for sb in range(S // 512):
                            nc.vector.tensor_copy(
                                dst[:, 512 * sb : 512 * (sb + 1)], ps[:, sb, :]
                            )

                ctx_pps = tc.tile_pool(name="proj_ps", bufs=2, space="PSUM")
                pps = ctx_pps.__enter__()
                # V = transpose(V^T) per 128-chunk
                for c in range(SKC):
                    pt = pps.tile([128, 128], bf16, tag="vtr")
                    nc.tensor.transpose(
                        pt[:], vT_sb[:, 128 * c : 128 * (c + 1)], ident[:]
                    )
                    nc.vector.tensor_copy(v_sb[:, c, :], pt[:])

                # Q^T per head (queries are columns 0..SQ-1 of the rolled xt)
                for h in range(NH):
                    ps = pps.tile([128, SQ], f32, tag="projq")
                    for sqh in range(2):
                        for c in range(HC):
                            nc.tensor.matmul(
                                ps[:, 512 * sqh : 512 * (sqh + 1)],
                                wq_sb[:, c, D * h : D * (h + 1)],
                                xt_sb[:, c, 512 * sqh : 512 * (sqh + 1)],
                                start=(c == 0),
                                stop=(c == HC - 1),
                            )
                    nc.vector.tensor_copy(qT_sb[:, h, :], ps[:])
                ctx_pps.__exit__(None, None, None)

            # ---------- phase 4: attention per head ----------
            with (
                tc.tile_pool(name="sc_ps", bufs=2, space="PSUM") as scp,
                tc.tile_pool(name="av_ps", bufs=1, space="PSUM") as avp,
                tc.tile_pool(name="rs_ps", bufs=2, space="PSUM") as rsp,
                tc.tile_pool(name="esb", bufs=2) as ep,
                tc.tile_pool(name="small", bufs=2) as smp,
            ):
                nc.sync.dma_start(
                    wo_sb[:], wot.ap().rearrange("(h p) n -> p h n", p=128)
                )

                def head_tail(h, av, rsh):
                    # normalization: broadcast rowsum over partitions with a
                    # rank-1 f32 matmul, fast reciprocal on DVE, multiply.
                    rs_sb = smp.tile([1, SQ], bf16, tag="rssb")
                    for sqh in range(2):
                        nc.vector.tensor_copy(
                            rs_sb[:, 512 * sqh : 512 * (sqh + 1)], rsh[sqh][0:1, :]
                        )
                    for sqh in range(2):
                        bc = rsp.tile([128, 512], f32, tag="rsbc")
                        nc.tensor.matmul(
                            bc[:],
                            ones_row[:],
                            rs_sb[:, 512 * sqh : 512 * (sqh + 1)],
                            start=True,
                            stop=True,
                        )
                        brec = smp.tile([128, 512], f32, tag="brec")
                        nc.vector.reciprocal_approx_fast(brec[:], bc[:])
                        nc.vector.tensor_mul(
                            outT_sb[:, h, 512 * sqh : 512 * (sqh + 1)],
                            av[:, 512 * sqh : 512 * (sqh + 1)],
                            brec[:],
                        )

                prev_tail = None
                for h in range(NH):
                    e_sb = ep.tile([128, SKC, SQ], bf16, tag="e")
                    av = avp.tile([128, SQ], f32, tag="av")
                    # rowsum halves at partition 0 of two one-bank tiles; the
                    # same pool slots are later reused for the broadcast tiles
                    # (same tag -> shared slots, disjoint lifetimes).
                    rsh = [
                        rsp.tile([128, 512], f32, tag="rsbc", name=f"rs{i}")
                        for i in range(2)
                    ]
                    def emit_scores(c):
                        # scores^T chunk [Sk 128, Sq 1024] = (K^T slice)^T Q^T
                        sc = scp.tile([128, SQ], f32, tag="sc")
                        for sqh in range(2):
                            nc.tensor.matmul(
                                sc[:, 512 * sqh : 512 * (sqh + 1)],
                                kT_sb[:, 128 * c : 128 * (c + 1)],
                                qT_sb[:, h, 512 * sqh : 512 * (sqh + 1)],
                                start=True,
                                stop=True,
                            )
                        # e = exp(scale * scores + mask_bias)  (bias is per-Sk
                        # = per-partition, exactly what ACT bias supports)
                        nc.scalar.activation(
                            e_sb[:, c, :],
                            sc[:],
                            Exp,
                            bias=bias_sb[:, c : c + 1],
                            scale=SCALE,
                        )

                    def emit_rs_av(c):
                        # rowsum halves accumulate at partition 0; AV
                        # accumulates over all chunks.
                        for sqh in range(2):
                            nc.tensor.matmul(
                                rsh[sqh][0:1, :],
                                ones_col[:],
                                e_sb[:, c, 512 * sqh : 512 * (sqh + 1)],
                                start=(c == 0),
                                stop=(c == SKC - 1),
                            )
                        for sqh in range(2):
                            nc.tensor.matmul(
                                av[:, 512 * sqh : 512 * (sqh + 1)],
                                v_sb[:, c, :],
                                e_sb[:, c, 512 * sqh : 512 * (sqh + 1)],
                                start=(c == 0),
                                stop=(c == SKC - 1),
                            )

                    # keep scores+exp one chunk ahead of rowsum/AV so the PE
                    # always has independent score matmuls to run while the
                    # current chunk's exp resolves on the scalar engine.
                    emit_scores(0)
                    for c in range(SKC):
                        if c + 1 < SKC:
                            emit_scores(c + 1)
                        emit_rs_av(c)

                    # software pipelining: emit the previous head's
                    # normalization tail AFTER this head's matmul stream, so
                    # the tensor engine has dense work while the tail's
                    # DVE/PE chain resolves.
                    if prev_tail is not None:
                        head_tail(*prev_tail)
                    prev_tail = (h, av, rsh)
                head_tail(*prev_tail)

            # ---------- phase 5: O-projection (partial; host sums groups) ----
            with (
                tc.tile_pool(name="y_ps", bufs=2, space="PSUM") as yp,
                tc.tile_pool(name="y_sb", bufs=3) as ysb,
            ):
                for t in range(SQ // 128):
                    for nb in range(HIDDEN // 512):
                        ps = yp.tile([128, 512], f32, tag="y")
                        for h in range(NH):
                            nc.tensor.matmul(
                                ps[:],
                                outT_sb[:, h, 128 * t : 128 * (t + 1)],
                                wo_sb[:, h, 512 * nb : 512 * (nb + 1)],
                                start=(h == 0),
                                stop=(h == NH - 1),
                            )
                        ysb_t = ysb.tile([128, 512], f32, tag="ysb")
                        nc.vector.tensor_copy(ysb_t[:], ps[:])
                        nc.sync.dma_start(
                            y[128 * t : 128 * (t + 1), 512 * nb : 512 * (nb + 1)],
                            ysb_t[:],
                        )

    nc.compile()
    return nc


def _get_nc():
    if "nc" not in _NC_CACHE:
        _NC_CACHE["nc"] = _build_nc()
    return _NC_CACHE["nc"]


def kernel(hidden_states, attention_mask, Wq, Wk, Wv, Wo):
    global last_results
    from concourse.bass_utils import run_bass_kernel_spmd

    bf = ml_dtypes.bfloat16
    hidden_states = np.asarray(hidden_states, dtype=np.float32)
    attention_mask = np.asarray(attention_mask, dtype=np.float32)
    Wq = np.asarray(Wq, dtype=np.float32)
    Wk = np.asarray(Wk, dtype=np.float32)
    Wv = np.asarray(Wv, dtype=np.float32)
    Wo = np.asarray(Wo, dtype=np.float32)

    nc = _get_nc()

    in_maps = []
    cores = []
    for b in range(2):
        xt_full = np.ascontiguousarray(hidden_states[b].T).astype(bf)  # [H, S]
        bias_full = ((1.0 - attention_mask[b]) * -10000.0).astype(np.float32)
        for hh in range(2):
            wqt = np.ascontiguousarray(
                Wq[NH * D * hh : NH * D * (hh + 1), :].T
            ).astype(bf)
            wkt = np.ascontiguousarray(Wk[D * hh : D * (hh + 1), :].T).astype(bf)
            wvt = np.ascontiguousarray(Wv[D * hh : D * (hh + 1), :].T).astype(bf)
            wot = np.ascontiguousarray(
                Wo[:, NH * D * hh : NH * D * (hh + 1)].T
            ).astype(bf)
            for th in range(2):
                # roll tokens so this core's queries are columns 0..SQ-1
                r = th * SQ
                xt_r = np.ascontiguousarray(
                    np.concatenate([xt_full[:, r:], xt_full[:, :r]], axis=1)
                )
                bias_r = np.concatenate([bias_full[r:], bias_full[:r]])
                biasd = np.ascontiguousarray(
                    bias_r.reshape(SKC, 128).T
                ).astype(np.float32)
                in_maps.append(
                    {
                        "xt": xt_r,
                        "wqt": wqt,
                        "wkt": wkt,
                        "wvt": wvt,
                        "wot": wot,
                        "biasd": biasd,
                    }
                )
                cores.append((b, hh, th))

    res = run_bass_kernel_spmd(nc, in_maps, core_ids=list(range(8)))
    last_results = res

    out = np.zeros((B, S, HIDDEN), dtype=np.float32)
    for (b, hh, th), r in zip(cores, res.results):
        out[b, th * SQ : (th + 1) * SQ, :] += r["y"]
    return out
